# revision 11
# baseline (speedup 1.0000x reference)
"""BertAttention (relative_key_query) Trainium2 Bass kernel.

Sharding: data-parallel over batch — core i computes batch element i fully
(no collectives).  Weights + distance table replicated.

Per-core algorithm (S=1024, H=16, D=64, E=1024):
  - Attention math in "transposed" orientation: scoresT[r, l], so the PV
    matmul consumes probsT directly and the softmax denominator comes free
    as a 65th row of the PV matmul (ones-augmented V).
  - Relative-position terms are Toeplitz gathers:
      rel_q[l, r]  = q[l] . E[l - r + S - 1]
      rel_k[l, r]  = k[r] . E[l - r + S - 1]
    For each 128-row block, matmul q (k) against an (S+128)-wide window of
    the (reversed) distance table, stage [128, S+128] to DRAM, and read it
    back with a stride-(S+127) access pattern — the diagonal gather becomes
    a plain strided DMA.  rel_kT lands directly in [r, l] orientation
    (DVE-added into scores); rel_q lands in [l, r] orientation and is
    PE-transpose-accumulated into the scoresT PSUM tile.
  - Softmax without max-subtraction (scores are O(1); exp exact, and the
    1/sqrt(D) scale plus q-bias are folded into Wq / the distance table on
    the host).
  - Output projection + residual + LayerNorm in normal orientation.
"""

import sys
from contextlib import ExitStack

sys.path.insert(0, "/opt/trn_rl_repo")

import numpy as np

import concourse.bass as bass
import concourse.tile as tile
from concourse import bacc, mybir
from concourse.masks import make_identity

F32 = mybir.dt.float32
F32R = mybir.dt.float32r
F16 = mybir.dt.float16
AF = mybir.ActivationFunctionType

S = 1024  # sequence length (== max_position_embeddings)
H = 16  # heads
D = 64  # head dim
E = H * D  # hidden
N_CORES = 8
EPS = 1e-12


def skew_ap(t, s):
    """Strided view of a [128, s+128] staged block realizing
    rel[lam, r] = blk[lam, r + 127 - lam]."""
    return bass.AP(t, 127, [[s + 127, 128], [1, s]])


def rep_ap(t, parts, n):
    """DMA source AP replicating a [1, n] DRAM tensor across partitions."""
    return bass.AP(t, 0, [[0, parts], [1, n]])


def build_program(s=S, h_heads=H, d=D, use_f32r=True, stage_f16=True,
                  lq=256):
    e = h_heads * d
    assert d == 64 and e % 128 == 0 and s % lq == 0 and lq % 128 == 0
    nblk = s // 128  # 128-row blocks in S
    nch = e // 128  # 128-row chunks in E
    nq = s // lq  # l-quarters
    bpq = lq // 128  # blocks per quarter
    win = s + 128  # staged window width (incl. 1 padded col)
    tabw = 2 * s  # table tiles: 2S-1 real cols + 1 zero col
    mmdt = F32R if use_f32r else F32
    stgdt = F16 if stage_f16 else F32

    nc = bacc.Bacc(None)

    # ---- external I/O ----
    hsT_d = nc.dram_tensor("hsT", [e, s], F32, kind="ExternalInput")
    hsr_d = nc.dram_tensor("hs_res", [s, e], F32, kind="ExternalInput")
    wq_d = nc.dram_tensor("wq8", [e, e], F32, kind="ExternalInput")
    wk_d = nc.dram_tensor("wk", [e, e], F32, kind="ExternalInput")
    wv_d = nc.dram_tensor("wv", [e, e], F32, kind="ExternalInput")
    wo_d = nc.dram_tensor("wo", [e, e], F32, kind="ExternalInput")
    et8_d = nc.dram_tensor("et8", [d, 2 * s - 1], F32, kind="ExternalInput")
    ehat_d = nc.dram_tensor("ehatT", [d, 2 * s - 1], F32, kind="ExternalInput")
    bq_d = nc.dram_tensor("bq8", [nch, 128], F32, kind="ExternalInput")
    bk_d = nc.dram_tensor("bk", [nch, 128], F32, kind="ExternalInput")
    bv_d = nc.dram_tensor("bv", [1, e], F32, kind="ExternalInput")
    gam_d = nc.dram_tensor("gamma", [1, e], F32, kind="ExternalInput")
    bet_d = nc.dram_tensor("beta", [1, e], F32, kind="ExternalInput")
    out_d = nc.dram_tensor("out", [s, e], F32, kind="ExternalOutput")

    # ---- internal DRAM staging (distinct per (term, head, block)) ----
    wstg = [[nc.dram_tensor(f"wstg_{h}_{b}", [128, win], stgdt)
             for b in range(nblk)] for h in range(h_heads)]
    dend = [[nc.dram_tensor(f"dend_{h}_{v}", [1, lq], F32)
             for v in range(nq)] for h in range(h_heads)]
    ystg = [[nc.dram_tensor(f"ystg_{h}_{b}", [128, win], stgdt)
             for b in range(nblk)] for h in range(h_heads)]

    with tile.TileContext(nc) as tc, ExitStack() as stk:
        consts = stk.enter_context(tc.tile_pool(name="consts", bufs=1))
        v_pool = stk.enter_context(tc.tile_pool(name="vp", bufs=1))
        ctx_pool = stk.enter_context(tc.tile_pool(name="ctxp", bufs=1))
        qk_stk = ExitStack()
        qk_pool = qk_stk.enter_context(tc.tile_pool(name="qkp", bufs=1))
        ps512 = stk.enter_context(tc.tile_pool(name="ps512", bufs=3, space="PSUM"))
        ps_sc = stk.enter_context(tc.tile_pool(name="ps_sc", bufs=2, space="PSUM"))
        ps_ctx = stk.enter_context(tc.tile_pool(name="ps_ctx", bufs=2, space="PSUM"))

        # ---------- constants ----------
        et8 = consts.tile([128, tabw], F32, tag="et8", name="et8")
        ehat = consts.tile([128, tabw], F32, tag="ehat", name="ehat")
        nc.vector.memset(et8[:, tabw - 1:tabw], 0.0)
        nc.vector.memset(ehat[:, tabw - 1:tabw], 0.0)
        for half in range(2):
            p0 = half * 64
            nc.gpsimd.dma_start(et8[p0:p0 + 64, 0:2 * s - 1], et8_d[:, :])
            nc.gpsimd.dma_start(ehat[p0:p0 + 64, 0:2 * s - 1], ehat_d[:, :])
        ident = consts.tile([128, 128], F32, tag="ident", name="ident")
        make_identity(nc, ident)
        bq_sb = consts.tile([128, nch], F32, tag="bq", name="bq")
        bk_sb = consts.tile([128, nch], F32, tag="bk", name="bk")
        for m in range(nch):
            nc.gpsimd.dma_start(bq_sb[:, m:m + 1], bq_d[m, :])
            nc.gpsimd.dma_start(bk_sb[:, m:m + 1], bk_d[m, :])
        eps_sb = consts.tile([128, 1], F32, tag="eps", name="eps")
        nc.vector.memset(eps_sb, EPS)

        # ---------- persistent activations ----------
        qT = [qk_pool.tile([128, s], F32, tag=f"qT{c}", name=f"qT{c}") for c in range(nch)]
        kT = [qk_pool.tile([128, s], F32, tag=f"kT{c}", name=f"kT{c}") for c in range(nch)]
        vaug = [v_pool.tile([128, h_heads * 65], F32, tag=f"v{r}", name=f"v{r}")
                for r in range(nblk)]

        # ---------- stage A/B: load hs^T + weights, project q/k/v ----------
        with tc.tile_pool(name="ab", bufs=1) as ab:
            bv_sb = ab.tile([128, e], F32, tag="bv", name="bv")
            nc.gpsimd.dma_start(bv_sb, rep_ap(bv_d, 128, e))
            hsT = [ab.tile([128, s], F32, tag=f"hsT{c}", name=f"hsT{c}") for c in range(nch)]
            for c in range(nch):
                nc.gpsimd.dma_start(hsT[c], hsT_d[c * 128:(c + 1) * 128, :])

            nfree = min(512, s)
            nspl = s // nfree

            def project_T(w_dram, bias_sb, dest):
                for m in range(nch):
                    wcol = ab.tile([128, nch, 128], F32, tag="wcol",
                                   name="wcol", bufs=2)
                    for c in range(nch):
                        nc.gpsimd.dma_start(
                            wcol[:, c, :],
                            w_dram[c * 128:(c + 1) * 128, m * 128:(m + 1) * 128])
                    for n in range(nspl):
                        ps = ps512.tile([128, 512], F32, tag="ps512", name="ps512")
                        for c in range(nch):
                            nc.tensor.matmul(
                                ps[:, :nfree],
                                lhsT=wcol[:, c, :].bitcast(mmdt),
                                rhs=hsT[c][:, n * nfree:(n + 1) * nfree].bitcast(mmdt),
                                start=(c == 0), stop=(c == nch - 1))
                        nc.scalar.activation(
                            dest[m][:, n * nfree:(n + 1) * nfree],
                            ps[:, :nfree], AF.Identity, bias=bias_sb[:, m:m + 1])

            project_T(wq_d, bq_sb, qT)
            project_T(wk_d, bk_sb, kT)

            # v in normal orientation [r, e'] with a ones column per head
            nefree = min(512, e)
            nespl = e // nefree
            hpn = nefree // 64  # heads per n-slice
            for n in range(nespl):
                wvb = ab.tile([128, nch, nefree], F32, tag="wvb", name="wvb")
                for c in range(nch):
                    nc.gpsimd.dma_start(
                        wvb[:, c, :],
                        wv_d[c * 128:(c + 1) * 128,
                             n * nefree:(n + 1) * nefree])
                for r in range(nblk):
                    ps = ps512.tile([128, 512], F32, tag="ps512", name="ps512")
                    for c in range(nch):
                        nc.tensor.matmul(
                            ps[:, :nefree],
                            lhsT=hsT[c][:, r * 128:(r + 1) * 128].bitcast(mmdt),
                            rhs=wvb[:, c, :].bitcast(mmdt),
                            start=(c == 0), stop=(c == nch - 1))
                    vv = vaug[r].rearrange("p (h x) -> p h x", x=65)
                    nc.vector.tensor_add(
                        vv[:, n * hpn:(n + 1) * hpn, 0:64],
                        ps[:, :nefree].rearrange("p (h x) -> p h x", x=64),
                        bv_sb[:, n * nefree:(n + 1) * nefree]
                        .rearrange("p (h x) -> p h x", x=64))
            for r in range(nblk):
                vv = vaug[r].rearrange("p (h x) -> p h x", x=65)
                nc.vector.memset(vv[:, :, 64:65], 1.0)

        # ---------- stage C: per-head attention ----------
        ctxT = [ctx_pool.tile([128, s], F32, tag=f"cx{c}", name=f"cx{c}")
                for c in range(nch)]
        cstk = ExitStack()
        stg_ev = cstk.enter_context(tc.tile_pool(name="stg_ev", bufs=2))
        relk_p = cstk.enter_context(tc.tile_pool(name="relk", bufs=1))
        relq_p = cstk.enter_context(tc.tile_pool(name="relq", bufs=1))
        prob_p = cstk.enter_context(tc.tile_pool(name="prob", bufs=1))
        tmp_p = cstk.enter_context(tc.tile_pool(name="tmp", bufs=2))
        den_p = cstk.enter_context(tc.tile_pool(name="den", bufs=2))
        cxb_p = cstk.enter_context(tc.tile_pool(name="cxb", bufs=2))

        nwin = [(j * 512, min(512, win - j * 512))
                for j in range((win + 511) // 512)]

        for h in range(h_heads):
            ch, base = (h * 64) // 128, (h * 64) % 128
            qs = qT[ch][base:base + 64, :]
            ks = kT[ch][base:base + 64, :]

            # --- stage W (q8 @ Ehat window) and Y (k @ E/8 window) ---
            for b in range(nblk):
                c_lo = (s - 128) - b * 128
                wsb = stg_ev.tile([128, win], stgdt, tag="wsb", name="wsb")
                ysb = stg_ev.tile([128, win], stgdt, tag="ysb", name="ysb")
                for (n0, nw) in nwin:
                    psw = ps512.tile([128, 512], F32, tag="ps512", name="ps512")
                    nc.tensor.matmul(
                        psw[:, :nw],
                        lhsT=qs[:, b * 128:(b + 1) * 128].bitcast(mmdt),
                        rhs=ehat[base:base + 64,
                                 c_lo + n0:c_lo + n0 + nw].bitcast(mmdt),
                        start=True, stop=True)
                    nc.scalar.activation(wsb[:, n0:n0 + nw], psw[:, :nw],
                                         AF.Copy)
                    psy = ps512.tile([128, 512], F32, tag="ps512", name="ps512")
                    nc.tensor.matmul(
                        psy[:, :nw],
                        lhsT=ks[:, b * 128:(b + 1) * 128].bitcast(mmdt),
                        rhs=et8[base:base + 64,
                                c_lo + n0:c_lo + n0 + nw].bitcast(mmdt),
                        start=True, stop=True)
                    nc.vector.tensor_copy(ysb[:, n0:n0 + nw], psy[:, :nw])
                nc.gpsimd.dma_start(wstg[h][b][:, :], wsb)
                nc.gpsimd.dma_start(ystg[h][b][:, :], ysb)

            # --- rel_kT skew reads (whole head, f16) ---
            relk = []
            for b in range(nblk):
                t = relk_p.tile([128, s], stgdt, tag=f"rk{b}", name=f"rk{b}")
                nc.gpsimd.dma_start(t, skew_ap(ystg[h][b], s))
                relk.append(t)

            # --- scores + softmax + PV, per l-quarter ---
            for v_i in range(nq):
                lsl = slice(v_i * lq, v_i * lq + lq)
                relq = []
                for j in range(bpq):
                    b = v_i * bpq + j
                    t = relq_p.tile([128, s], F32, tag=f"rq{j}", name=f"rq{j}")
                    nc.gpsimd.dma_start(t, skew_ap(wstg[h][b], s))
                    relq.append(t)

                probs = []
                for r in range(nblk):
                    ps = ps_sc.tile([128, lq], F32, tag="sc", name="sc")
                    nc.tensor.matmul(
                        ps,
                        lhsT=ks[:, r * 128:(r + 1) * 128].bitcast(mmdt),
                        rhs=qs[:, lsl].bitcast(mmdt),
                        start=True, stop=False)
                    for j in range(bpq):
                        nc.tensor.matmul(
                            ps[:, j * 128:(j + 1) * 128],
                            lhsT=relq[j][:, r * 128:(r + 1) * 128],
                            rhs=ident,
                            is_transpose=True,
                            start=False, stop=(j == bpq - 1))
                    ssb = tmp_p.tile([128, lq], F32, tag="ssb", name="ssb")
                    nc.vector.tensor_add(ssb, ps, relk[r][:, lsl])
                    pb = prob_p.tile([128, lq], F32, tag=f"pb{r}", name=f"pb{r}")
                    nc.scalar.activation(pb, ssb, AF.Exp)
                    probs.append(pb)

                pc = ps_ctx.tile([65, lq], F32, tag="ctx", name="ctx")
                for r in range(nblk):
                    nc.tensor.matmul(
                        pc,
                        lhsT=vaug[r][:, h * 65:(h + 1) * 65].bitcast(mmdt),
                        rhs=probs[r].bitcast(mmdt),
                        start=(r == 0), stop=(r == nblk - 1))
                # reciprocal of denominator (psum row 64, stays on part 64)
                rden = den_p.tile([128, lq], F32, tag="rden", name="rden")
                nc.vector.reciprocal(rden[64:65, :], pc[64:65, :])
                # replicate across partitions 0..63 via a DRAM bounce
                nc.gpsimd.dma_start(dend[h][v_i][:, :], rden[64:65, :])
                rrep = den_p.tile([128, lq], F32, tag="rrep", name="rrep")
                nc.gpsimd.dma_start(rrep[0:64, :], rep_ap(dend[h][v_i], 64, lq))
                if base == 0:
                    nc.vector.tensor_mul(ctxT[ch][0:64, lsl], pc[0:64, :],
                                         rrep[0:64, :])
                else:
                    cb = cxb_p.tile([128, lq], F32, tag="cb", name="cb")
                    nc.vector.tensor_mul(cb[0:64, :], pc[0:64, :],
                                         rrep[0:64, :])
                    nc.gpsimd.dma_start(ctxT[ch][64:128, lsl], cb[0:64, :])

        cstk.close()
        qk_stk.close()

        # ---------- stage D: out projection + residual + LayerNorm ----------
        with tc.tile_pool(name="dstage", bufs=1) as dp, \
                tc.tile_pool(name="dtmp", bufs=2) as dtmp, \
                tc.tile_pool(name="dst", bufs=4) as dst:
            wo_sb = [dp.tile([128, e], F32, tag=f"wo{c}", name=f"wo{c}") for c in range(nch)]
            for c in range(nch):
                nc.gpsimd.dma_start(wo_sb[c], wo_d[c * 128:(c + 1) * 128, :])
            gam_sb = dp.tile([128, e], F32, tag="gam", name="gam")
            bet_sb = dp.tile([128, e], F32, tag="bet", name="bet")
            nc.gpsimd.dma_start(gam_sb, rep_ap(gam_d, 128, e))
            nc.gpsimd.dma_start(bet_sb, rep_ap(bet_d, 128, e))
            nefree = min(512, e)
            nespl = e // nefree
            for m in range(s // 128):
                osb = dtmp.tile([128, e], F32, tag="osb", name="osb")
                hres = dtmp.tile([128, e], F32, tag="hres", name="hres")
                nc.gpsimd.dma_start(hres, hsr_d[m * 128:(m + 1) * 128, :])
                for n in range(nespl):
                    ps = ps512.tile([128, 512], F32, tag="ps512", name="ps512")
                    for c in range(nch):
                        nc.tensor.matmul(
                            ps[:, :nefree],
                            lhsT=ctxT[c][:, m * 128:(m + 1) * 128].bitcast(mmdt),
                            rhs=wo_sb[c][:, n * nefree:(n + 1) * nefree].bitcast(mmdt),
                            start=(c == 0), stop=(c == nch - 1))
                    nsl = slice(n * nefree, (n + 1) * nefree)
                    nc.vector.tensor_add(osb[:, nsl], ps[:, :nefree],
                                         hres[:, nsl])
                # layernorm over free dim e
                nsub = (e + 511) // 512
                gs = min(512, e)
                stats = dst.tile([128, nsub, 6], F32, tag="st", name="st")
                for g in range(nsub):
                    nc.vector.bn_stats(stats[:, g, :],
                                       osb[:, g * gs:(g + 1) * gs])
                mv = dst.tile([128, 2], F32, tag="mv", name="mv")
                nc.vector.bn_aggr(mv, stats)
                sd = dst.tile([128, 1], F32, tag="sd", name="sd")
                nc.scalar.activation(sd, mv[:, 1:2], AF.Sqrt, bias=eps_sb)
                rsig = dst.tile([128, 1], F32, tag="rs", name="rs")
                nc.vector.reciprocal(rsig, sd)
                tnorm = dtmp.tile([128, e], F32, tag="tn", name="tn")
                nc.vector.tensor_scalar(tnorm, osb, mv[:, 0:1], rsig,
                                        op0=mybir.AluOpType.subtract,
                                        op1=mybir.AluOpType.mult)
                nc.vector.tensor_mul(tnorm, tnorm, gam_sb)
                nc.vector.tensor_add(tnorm, tnorm, bet_sb)
                nc.gpsimd.dma_start(out_d[m * 128:(m + 1) * 128, :], tnorm)


    nc.finalize()
    return nc


def host_prep(hidden_states, Wq, bq, Wk, bk, Wv, bv, dist_emb, Wo, bo,
              ln_gamma, ln_beta, s=S, h_heads=H, d=D):
    """Build the per-core input maps (core i handles batch i)."""
    e = h_heads * d
    nch = e // 128
    B = hidden_states.shape[0]
    hidden_states = np.asarray(hidden_states, np.float32)
    scale = np.float32(1.0 / np.sqrt(d))
    dist_emb = np.asarray(dist_emb, np.float32)
    shared = {
        "wq8": np.ascontiguousarray(np.asarray(Wq, np.float32) * scale),
        "wk": np.ascontiguousarray(np.asarray(Wk, np.float32)),
        "wv": np.ascontiguousarray(np.asarray(Wv, np.float32)),
        "wo": np.ascontiguousarray(np.asarray(Wo, np.float32)),
        "et8": np.ascontiguousarray(dist_emb.T * scale),
        "ehatT": np.ascontiguousarray(dist_emb[::-1].T),
        "bq8": np.ascontiguousarray(
            (np.asarray(bq, np.float32) * scale).reshape(nch, 128)),
        "bk": np.ascontiguousarray(np.asarray(bk, np.float32).reshape(nch, 128)),
        "bv": np.asarray(bv, np.float32).reshape(1, e),
        "gamma": np.asarray(ln_gamma, np.float32).reshape(1, e),
        "beta": np.asarray(ln_beta, np.float32).reshape(1, e),
    }
    bo = np.asarray(bo, np.float32)
    in_maps = []
    for b in range(B):
        hs = np.ascontiguousarray(hidden_states[b])
        m = dict(shared)
        m["hsT"] = np.ascontiguousarray(hs.T)
        m["hs_res"] = hs + bo[None, :]
        in_maps.append(m)
    return in_maps


_CACHE = {}


def _get_program():
    import os
    if "nc" not in _CACHE:
        _CACHE["nc"] = build_program(
            use_f32r=os.environ.get("F32R", "1") == "1")
    return _CACHE["nc"]


def kernel(**inputs):
    from concourse.bass_utils import run_bass_kernel_spmd
    nc = _get_program()
    in_maps = host_prep(**inputs)
    res = run_bass_kernel_spmd(nc, in_maps, list(range(N_CORES)))
    out = np.stack([res.results[i]["out"] for i in range(N_CORES)], axis=0)
    return out.astype(np.float32)
